# revision 1
# baseline (speedup 1.0000x reference)
"""TopoEncoder Trainium2 kernel v3 (8 NeuronCores, data-parallel over batch).

Differences vs v1 baseline (104-113us):
  - ONE CC collective (AllGather of the local max(d2) scalar) instead of
    warmup-AllReduce + real-AllReduce. Measurements show the first op on
    the CC queue completes at ~74us after kernel entry regardless of
    trigger time or payload, so the second serialized mesh op (~10us) and
    the DRAM bounce of the min side are pure waste. The global max lands
    ~76us, just as the FW closure + extraction finish.
  - Global min is hardcoded: the reference normalizes over the whole
    distance tensor whose diagonal is exactly sqrt(1e-12)=1e-6, while
    gmax is O(0.5) -- so (d-gmin)/(gmax-gmin) == d/gmax to ~1.4e-6 rel.
  - Every post-collective op except the final normalize multiply runs on
    GpSimd/Scalar/PE; the one Vector op (reciprocal) is pinned AFTER the
    extraction via a bypass-op data dependency on `deaths` (v1's scheduler
    hoisted collective-dependent vector ops into the FW stream and
    stalled the in-order vector queue for 19.5us).
  - Structure-element layer runs on 128 partitions (partition (h,b) owns
    e-half h of sample b) -- halves the tail's DVE time. Exp ACT table is
    preloaded off-path so the tail pays no table switch.
  - Cross-partition reduce via gpsimd tensor_reduce(axis=XYZWC) (drops
    the id64 transpose matmul input of v1).
"""

from contextlib import ExitStack

import numpy as np

import bass_rust
import concourse.bass as bass
import concourse.tile as tile
from concourse import mybir
from concourse.bass_utils import run_bass_kernel_spmd

N_CORES = 8
B = 64          # samples per core
C, T, V, E = 3, 128, 25, 64
VV = V * V
NT = V - 1      # deaths per sample (24)
E2 = E // 2     # e-half per partition group (32)
DT = mybir.dt.float32


def _split_excess_waits(nc, cap=1):
    """The walrus build in this env rejects instructions carrying more than
    ~2 semaphore-wait commands. Move excess waits onto same-engine NOPs
    inserted immediately before the offending instruction."""
    n_split = 0
    for bb in nc.main_func.blocks:
        insts = bb.instructions
        i = 0
        while i < len(insts):
            ins = insts[i]
            si = ins.sync_info
            waits = list(si.on_wait) if si and si.on_wait else []
            if len(waits) > cap:
                extra, keep = waits[:-cap], waits[-cap:]
                ins.sync_info = mybir.SyncInfo(
                    on_wait=keep, on_update=list(si.on_update or [])
                )
                for j, w in enumerate(extra):
                    nop = bass_rust.InstNoOp(
                        name=f"I-wsplit-{n_split}-{j}",
                        engine=ins.engine,
                        sync_info=mybir.SyncInfo(on_wait=[w], on_update=[]),
                    )
                    insts.insert(i, nop)
                    i += 1
                n_split += 1
            i += 1
    return n_split


def _insert_wait_nops(nc, target_name, waits):
    """Insert one NOP per (sem, val) wait immediately before the named
    instruction. Runs post-scheduling, same mechanism as the wait splitter."""
    for bb in nc.main_func.blocks:
        insts = bb.instructions
        for i, ins in enumerate(insts):
            if ins.name == target_name:
                for k, (sem, val) in enumerate(waits):
                    nop = bass_rust.InstNoOp(
                        name=f"I-rwait-{k}-{target_name}",
                        engine=ins.engine,
                        sync_info=mybir.SyncInfo(on_wait=[], on_update=[]),
                    )
                    bass_rust.wait_op(nop, sem, val, "sem-ge", True)
                    insts.insert(i, nop)
                return
    raise KeyError(f"instruction {target_name} not found")


def _build_program():
    A = mybir.AluOpType
    ACT = mybir.ActivationFunctionType
    nc = bass.Bass("TRN2", debug=False, num_devices=N_CORES)

    x_in = nc.dram_tensor("x", [B, C, T, V], DT, kind="ExternalInput").ap()
    # csH rows: h -> [c2_h(32) | s2_h(32) | s1_h(32) | c1_h(32)]
    csH_in = nc.dram_tensor("csH", [2, 4 * E2], DT, kind="ExternalInput").ap()
    pm_in = nc.dram_tensor("pm", [128, B], DT, kind="ExternalInput").ap()
    ut_in = nc.dram_tensor("ut", [1, VV], DT, kind="ExternalInput").ap()
    rep_in = nc.dram_tensor("rep", [B, 128], DT, kind="ExternalInput").ap()
    sel_in = nc.dram_tensor("sel", [2, 128], DT, kind="ExternalInput").ap()
    out_d = nc.dram_tensor("out", [B, E], DT, kind="ExternalOutput").ap()

    with tile.TileContext(nc, num_cores=N_CORES) as tc, ExitStack() as ctx:
        sb = ctx.enter_context(tc.tile_pool(name="sb", bufs=1))
        work = ctx.enter_context(tc.tile_pool(name="work", bufs=2))
        psum = ctx.enter_context(tc.tile_pool(name="psum", bufs=1, space="PSUM"))
        dram = ctx.enter_context(tc.tile_pool(name="dram", bufs=1, space="DRAM"))

        ones1 = sb.tile([1, 128], DT)
        nc.vector.memset(ones1[:], 1.0)

        # ---- x DMA first: partition p = t2*64 + b, free = (c, t64, v) ----
        # two t64-half tiles so the add-tree overlaps the second half's DMA;
        # both HWDGE queues used; 3.2KB contiguous runs
        xa = sb.tile([128, C, T // 4, V], DT)
        xb = sb.tile([128, C, T // 4, V], DT)
        nc.sync.dma_start(xa[0:B], x_in[:, :, 0:32, :])
        nc.scalar.dma_start(xa[B:128], x_in[:, :, 64:96, :])
        nc.sync.dma_start(xb[0:B], x_in[:, :, 32:64, :])
        nc.scalar.dma_start(xb[B:128], x_in[:, :, 96:128, :])

        # ---- small constant loads (HWDGE, few descriptors) ----
        pm_t = sb.tile([128, B], DT)
        nc.sync.dma_start(pm_t[:], pm_in[:])
        csh = sb.tile([2, 4 * E2], DT)
        nc.scalar.dma_start(csh[:], csH_in[:])
        utrow = sb.tile([1, VV], DT)
        nc.scalar.dma_start(utrow[:], ut_in[:])
        rep128 = sb.tile([B, 128], DT)
        nc.sync.dma_start(rep128[:], rep_in[:])
        sel2 = sb.tile([2, 128], DT)
        nc.scalar.dma_start(sel2[:], sel_in[:])
        eps = sb.tile([128, 1], DT)
        nc.vector.memset(eps[:], 1e-12)
        lmx = sb.tile([1, 1], DT)
        gsb8 = sb.tile([1, 8], DT)

        # ---- PE partition-broadcasts ----
        # upper-tri premask rows for all 64 samples
        utb = psum.tile([B, VV], DT)
        nc.tensor.matmul(out=utb[:, 0:512], lhsT=ones1[:, 0:B],
                         rhs=utrow[:, 0:512], start=True, stop=True)
        nc.tensor.matmul(out=utb[:, 512:VV], lhsT=ones1[:, 0:B],
                         rhs=utrow[:, 512:VV], start=True, stop=True)
        # per-half params onto 128 partitions: partition p gets half p//64
        prm2 = psum.tile([128, 4, E2], DT)
        nc.tensor.matmul(out=prm2[:], lhsT=sel2[:], rhs=csh[:],
                         start=True, stop=True)
        prm = sb.tile([128, 4, E2], DT)
        nc.vector.tensor_copy(prm[:], prm2[:])
        c2p = prm[:, 0, :]

        # derived parameter tiles: ub = s2^2, Ab = exp(-(s1*c1)^2)
        ub = sb.tile([128, E2], DT)
        nc.scalar.square(ub[:], prm[:, 1, :])
        m1t = sb.tile([128, E2], DT)
        nc.vector.tensor_tensor(out=m1t[:], in0=prm[:, 2, :], in1=prm[:, 3, :],
                                op=A.mult)
        nc.scalar.square(m1t[:], m1t[:])
        Ab = sb.tile([128, E2], DT)
        nc.scalar.activation(Ab[:], m1t[:], ACT.Exp, bias=0.0, scale=-1.0)

        # ---- mean over T: in-place add trees (DVE: c0-c1, GpSimd: c2),
        # then PE pair-matrix fold ----
        for xh in (xa, xb):
            for w in (16, 8, 4, 2, 1):
                nc.vector.tensor_tensor(
                    out=xh[:, 0:2, 0:w, :],
                    in0=xh[:, 0:2, 0:w, :],
                    in1=xh[:, 0:2, w : 2 * w, :],
                    op=A.add,
                )
                nc.gpsimd.tensor_tensor(
                    out=xh[:, 2, 0:w, :],
                    in0=xh[:, 2, 0:w, :],
                    in1=xh[:, 2, w : 2 * w, :],
                    op=A.add,
                )
        nc.vector.tensor_tensor(
            out=xa[:, 0:2, 0:1, :], in0=xa[:, 0:2, 0:1, :], in1=xb[:, 0:2, 0:1, :],
            op=A.add,
        )
        nc.gpsimd.tensor_tensor(
            out=xa[:, 2, 0:1, :], in0=xa[:, 2, 0:1, :], in1=xb[:, 2, 0:1, :],
            op=A.add,
        )
        ps_xm = psum.tile([B, C, V], DT)
        nc.tensor.matmul(out=ps_xm[:], lhsT=pm_t[:], rhs=xa[:, :, 0, :],
                         start=True, stop=True)
        xm = sb.tile([B, C, V], DT)
        nc.vector.tensor_copy(xm[:], ps_xm[:])

        # ---- distance matrix ----
        df = sb.tile([B, C, V, V], DT)
        xmb_i = xm.unsqueeze(-1).broadcast_to([B, C, V, V])
        xmb_j = xm.unsqueeze(2).broadcast_to([B, C, V, V])
        nc.vector.tensor_tensor(
            out=df[:, 0:2], in0=xmb_i[:, 0:2], in1=xmb_j[:, 0:2], op=A.subtract
        )
        nc.gpsimd.tensor_tensor(
            out=df[:, 2], in0=xmb_i[:, 2], in1=xmb_j[:, 2], op=A.subtract
        )
        nc.scalar.square(df[:, 0:2], df[:, 0:2])
        nc.vector.tensor_tensor(out=df[:, 2], in0=df[:, 2], in1=df[:, 2], op=A.mult)
        d2 = sb.tile([B, VV], DT)
        d23 = d2.rearrange("p (i j) -> p i j", i=V)
        nc.vector.tensor_tensor(out=d23[:], in0=df[:, 0], in1=df[:, 1], op=A.add)
        nc.vector.tensor_tensor(out=d23[:], in0=d23[:], in1=df[:, 2], op=A.add)
        dmat = sb.tile([B, VV], DT)
        nc.scalar.activation(dmat[:], d2[:], ACT.Sqrt, bias=eps[0:B, 0:1], scale=1.0)

        # ---- local max(d2) -> single AllGather (all on GpSimd/Scalar,
        # off the vector critical path) ----
        nc.gpsimd.tensor_reduce(out=lmx[:], in_=d2[:],
                                axis=mybir.AxisListType.XYZWC, op=A.max)
        cin = dram.tile([1, 1], DT)
        cout = dram.tile([1, 8], DT)
        nc.scalar.dma_start(cin[:], lmx[:])
        nc.gpsimd.collective_compute(
            "AllGather", A.bypass, replica_groups=[list(range(N_CORES))],
            ins=[cin.opt()], outs=[cout.opt()],
        )
        nc.scalar.dma_start(gsb8[:], cout[:])
        g2 = sb.tile([1, 1], DT)
        nc.gpsimd.tensor_reduce(out=g2[:], in_=gsb8[:],
                                axis=mybir.AxisListType.XYZWC, op=A.max)
        # gmax = sqrt(max d2 + eps) (Sqrt table already loaded for dmat);
        # then preload the Exp table so the structure tail pays no switch
        gmx = sb.tile([1, 1], DT)
        nc.scalar.activation(gmx[:], g2[:], ACT.Sqrt, bias=eps[0:1, 0:1], scale=1.0)
        exwarm = sb.tile([1, 1], DT)
        nc.scalar.activation(exwarm[:], g2[:], ACT.Exp, bias=0.0, scale=-1.0)

        # ---- premasked values (overlaps FW head) ----
        dut = sb.tile([B, VV], DT)
        nc.vector.tensor_tensor(out=dut[:], in0=dmat[:], in1=utb[:], op=A.mult)

        # ---- Floyd-Warshall min-max closure (in place, vector only) ----
        M = sb.tile([B, VV], DT)
        M3 = M.rearrange("p (i j) -> p i j", i=V)
        dm3 = dmat.rearrange("p (i j) -> p i j", i=V)
        fwt = sb.tile([B, V, V], DT)
        for k in range(V):
            src = dm3 if k == 0 else M3
            nc.vector.tensor_tensor(
                out=fwt[:],
                in0=src[:, :, k : k + 1].broadcast_to([B, V, V]),
                in1=src[:, k : k + 1, :].broadcast_to([B, V, V]),
                op=A.max,
            )
            nc.vector.tensor_tensor(out=M3[:], in0=src[:], in1=fwt[:], op=A.min)

        # ---- MST mask + masked upper-tri values ----
        mk = sb.tile([B, VV], DT)
        nc.vector.tensor_tensor(out=mk[:], in0=M[:], in1=dmat[:], op=A.is_ge)
        val = sb.tile([B, VV], DT)
        nc.vector.tensor_tensor(out=val[:], in0=mk[:], in1=dut[:], op=A.mult)

        # ---- extract 24 MST weights: 3 rounds of top-8 + match_replace ----
        deaths = sb.tile([B, NT], DT)
        mr1 = sb.tile([B, VV], DT)
        mr2 = sb.tile([B, VV], DT)
        nc.vector.max(deaths[:, 0:8], val[:])
        nc.vector.match_replace(mr1[:], deaths[:, 0:8], val[:], 0.0)
        nc.vector.max(deaths[:, 8:16], mr1[:])
        nc.vector.match_replace(mr2[:], deaths[:, 8:16], mr1[:], 0.0)
        nc.vector.max(deaths[:, 16:24], mr2[:])

        # ---- replicate deaths onto both partition halves, normalize ----
        # The vector queue is in-order: a collective-dependent vector op
        # scheduled early would stall the FW stream behind it. The bypass
        # op gives the reciprocal a data dependency on `deaths`, pinning
        # the whole chain after the extraction.
        deaths2 = psum.tile([128, NT], DT)
        nc.tensor.matmul(out=deaths2[:], lhsT=rep128[:], rhs=deaths[:],
                         start=True, stop=True)
        gmxd = sb.tile([1, 1], DT)
        nc.vector.tensor_tensor(out=gmxd[:], in0=gmx[:], in1=deaths[0:1, 0:1],
                                op=A.bypass)
        inv = sb.tile([1, 1], DT)
        nc.vector.reciprocal(inv[:], gmxd[:])
        invb = psum.tile([128, 1], DT)
        nc.tensor.matmul(out=invb[:], lhsT=ones1[:], rhs=inv[:],
                         start=True, stop=True)
        dn = sb.tile([128, NT], DT)
        nc.vector.tensor_scalar_mul(dn[:], deaths2[:], invb[:, 0:1])

        # ---- structure element layer on 128 partitions (e-half per h) ----
        S = sb.tile([128, E2], DT)
        ECH = 16
        for ch in range(E2 // ECH):
            e0 = ch * ECH
            t1 = work.tile([128, ECH, NT], DT, tag="t1")
            nc.vector.tensor_tensor(
                out=t1[:],
                in0=dn.unsqueeze(1).broadcast_to([128, ECH, NT]),
                in1=c2p[:, e0 : e0 + ECH].unsqueeze(-1).broadcast_to([128, ECH, NT]),
                op=A.subtract,
            )
            nc.scalar.square(t1[:], t1[:])
            nc.vector.tensor_tensor(
                out=t1[:],
                in0=t1[:],
                in1=ub[:, e0 : e0 + ECH].unsqueeze(-1).broadcast_to([128, ECH, NT]),
                op=A.mult,
            )
            fexp = work.tile([128, ECH, NT], DT, tag="fexp")
            nc.scalar.activation(fexp[:], t1[:], ACT.Exp, bias=0.0, scale=-1.0)
            nc.vector.tensor_reduce(
                out=S[:, e0 : e0 + ECH], in_=fexp[:], axis=mybir.AxisListType.X,
                op=A.add,
            )
        outt = sb.tile([128, E2], DT)
        nc.vector.tensor_tensor(out=outt[:], in0=S[:], in1=Ab[:], op=A.mult)
        nc.sync.dma_start(out_d[:, 0:E2], outt[0:B])
        nc.scalar.dma_start(out_d[:, E2:E], outt[B:128])

    _split_excess_waits(nc)
    return nc


_CACHE = {}


def _consts():
    # pair matrix: adds partition rows b and b+64 (the two T-halves) and
    # applies the 1/T mean scale
    pairmat = np.zeros((128, B), dtype=np.float32)
    for p in range(128):
        pairmat[p, p % B] = 1.0 / T
    ut = np.triu(np.ones((V, V), dtype=np.float32), k=1).reshape(1, VV)
    rep = np.zeros((B, 128), dtype=np.float32)
    for p in range(128):
        rep[p % B, p] = 1.0
    sel = np.zeros((2, 128), dtype=np.float32)
    sel[0, 0:B] = 1.0
    sel[1, B:128] = 1.0
    return pairmat, np.ascontiguousarray(ut), rep, sel


def _get_program():
    if "nc" not in _CACHE:
        _CACHE["nc"] = _build_program()
    return _CACHE["nc"]


def _run(x, centres, sharpness, **run_kwargs):
    nc = _get_program()
    xf = np.ascontiguousarray(x.reshape(-1, C, T, V)).astype(np.float32, copy=False)
    n_total = xf.shape[0]
    assert n_total == N_CORES * B, xf.shape
    c1, c2 = centres[:, 0], centres[:, 1]
    s1, s2 = sharpness[:, 0], sharpness[:, 1]
    csH = np.stack(
        [
            np.concatenate([c2[h * E2 : (h + 1) * E2], s2[h * E2 : (h + 1) * E2],
                            s1[h * E2 : (h + 1) * E2], c1[h * E2 : (h + 1) * E2]])
            for h in range(2)
        ],
        axis=0,
    ).astype(np.float32)
    pairmat, ut, rep, sel = _consts()
    in_maps = [
        {
            "x": np.ascontiguousarray(xf[i * B : (i + 1) * B]),
            "csH": np.ascontiguousarray(csH),
            "pm": pairmat,
            "ut": ut,
            "rep": rep,
            "sel": sel,
        }
        for i in range(N_CORES)
    ]
    res = run_bass_kernel_spmd(nc, in_maps, list(range(N_CORES)), **run_kwargs)
    out = np.concatenate([res.results[i]["out"] for i in range(N_CORES)], axis=0)
    return out, res


def kernel(x, centres, sharpness):
    out, _ = _run(np.asarray(x), np.asarray(centres), np.asarray(sharpness))
    return out

